# revision 18
# baseline (speedup 1.0000x reference)
"""Trainium2 Bass kernel for CrossModalFuzzyAttention.

Sharding: data-parallel over batch (B=8) across 8 NeuronCores, one batch
element per core.  Each core runs the full fused pipeline for its element:

  text_proj  = text @ Wtp + btp            [1024, 256]
  image_proj = image @ Wip + bip           [1024, 256]
  text_enh   = fuzzy_attn(text_proj, image_proj, image_proj)
  image_enh  = fuzzy_attn(image_proj, text_proj, text_proj)
  text_final = (1-f)*text  + f*(text_enh @ Wto + bto)
  image_final= (1-f)*image + f*(image_enh @ Wio + bio)

On-chip layout: activations are kept feature-major ("X^T", [feat, seq]) so
every weight matmul is  out^T = W.T @ X^T  with W as the stationary operand,
and biases land on the partition dim.  V is produced in natural [seq, feat]
layout (ones-augmented per head) so attention denominators come out
per-partition and normalization fuses into the PSUM evacuation.

Numerics: scores = <sigmoid, sigmoid>/8 in (0, 8), so softmax needs no max
subtraction (exp <= e^8).  Matmul operands are bf16 (fp32 matmul is 2x
slower on TRN2); the residual blend path stays fp32.  sigmoid(x) is
computed as 0.5*tanh(x/2)+0.5 because Tanh and Exp share one ACT table set
(Sigmoid does not - each set switch costs ~2.7us).
"""
import sys

try:
    import concourse  # noqa: F401
except ImportError:
    sys.path.insert(0, "/opt/trn_rl_repo")

import numpy as np

import concourse.bass as bass
import concourse.tile as tile
from concourse import masks, mybir
from concourse.bass_utils import run_bass_kernel_spmd

F32 = mybir.dt.float32
BF16 = mybir.dt.bfloat16
AF = mybir.ActivationFunctionType
ALU = mybir.AluOpType

# ---------------------------------------------------------------------------
# The walrus in this container encodes only a small number of sem waits per
# instruction and fails codegen with "Too many sync wait commands" otherwise
# (Tile attaches one wait per producer proc, which can be many).  Legalize
# after scheduling: move excess waits onto same-engine NOPs inserted just
# before the over-subscribed instruction.
# ---------------------------------------------------------------------------
import bass_rust as _br

_MAX_WAITS = 1


def _split_sync_waits(nc: "bass.Bass", max_waits: int = _MAX_WAITS) -> None:
    uid = 0
    for f in nc.m.functions:
        for bb in f.blocks:
            il = bb.instructions
            out = []
            changed = False
            for inst in il:
                si = inst.sync_info
                waits = list(si.on_wait) if si is not None and si.on_wait else []
                if len(waits) > max_waits:
                    keep = waits[-max_waits:]
                    excess = waits[:-max_waits]
                    for i in range(0, len(excess), max_waits):
                        grp = excess[i:i + max_waits]
                        nop = _br.InstNoOp(
                            name=f"waitnop-{uid}", engine=inst.engine
                        )
                        uid += 1
                        nop.sync_info = _br.SyncInfo(on_wait=grp, on_update=[])
                        out.append(nop)
                    inst.sync_info = _br.SyncInfo(
                        on_wait=keep, on_update=list(si.on_update or [])
                    )
                    changed = True
                out.append(inst)
            if changed:
                bb.instructions = out

# problem shapes (hardcoded per spec)
ST = 1024   # text seq
SI = 1024   # image seq
DT = 512    # text dim
DI = 768    # image dim
H = 256     # hidden
NH = 4      # heads
DH = 64     # head dim
VSTR = DH + 4  # per-head stride in augmented-V tile (64 V cols + ones + pad)


def _build(f: float) -> bass.Bass:
    """Build the single-core bass program.  f = sigmoid(fusion_weight)."""
    # f is folded into Wto/Wio/bto/bio and (1-f) into the resident copies of
    # the raw inputs, so the final blend is a single tensor_add:
    #   out = psum{=f*(enh@Wout+bout)} + rawS{=(1-f)*x}
    omf = 1.0 - f

    nc = bass.Bass()

    # ---- DRAM tensors -----------------------------------------------------
    text = nc.dram_tensor("text", [ST, DT], F32, kind="ExternalInput")
    image = nc.dram_tensor("image", [SI, DI], F32, kind="ExternalInput")
    dW = {}
    for name, shp in [
        ("Wtp", [DT, H]), ("Wip", [DI, H]), ("Wq", [H, H]), ("Wk", [H, H]),
        ("Wv", [H, H]), ("Wo", [H, H]), ("Wto", [H, DT]), ("Wio", [H, DI]),
    ]:
        dW[name] = nc.dram_tensor(name, shp, F32, kind="ExternalInput")
    dB = {}
    for name, n in [
        ("btp", H), ("bip", H), ("bq", H), ("bk", H), ("bv", H), ("bo", H),
        ("bto", DT), ("bio", DI),
    ]:
        dB[name] = nc.dram_tensor(name, [n], F32, kind="ExternalInput")
    ident_dram = nc.dram_tensor("ident128", [128, 128], BF16, kind="ExternalInput")
    textT_dram = nc.dram_tensor("textT", [DT, ST], BF16, kind="ExternalInput")
    imageT_dram = nc.dram_tensor("imageT", [DI, SI], BF16, kind="ExternalInput")
    text_out = nc.dram_tensor("text_final", [ST, DT], F32, kind="ExternalOutput")
    image_out = nc.dram_tensor("image_final", [SI, DI], F32, kind="ExternalOutput")

    with tile.TileContext(nc) as tc:
        _body(nc, tc, text, image, textT_dram, imageT_dram, dW, dB, ident_dram, text_out, image_out, f, omf)
    _split_sync_waits(nc)
    return nc


def _body(nc, tc, text, image, textT_dram, imageT_dram, dW, dB, ident_dram, text_out, image_out, f, omf):
    import contextlib
    ctx = contextlib.ExitStack()
    with ctx:
        consts = ctx.enter_context(tc.tile_pool(name="consts", bufs=1))
        wpool = ctx.enter_context(tc.tile_pool(name="wpool", bufs=1))
        wraw = ctx.enter_context(tc.tile_pool(name="wraw", bufs=3))
        rawin = ctx.enter_context(tc.tile_pool(name="rawin", bufs=1))
        ttp = ctx.enter_context(tc.tile_pool(name="ttp", bufs=1))
        ptp = ctx.enter_context(tc.tile_pool(name="ptp", bufs=1))
        qkp = ctx.enter_context(tc.tile_pool(name="qkp", bufs=1))
        vaugp = ctx.enter_context(tc.tile_pool(name="vaugp", bufs=1))
        sigp = ctx.enter_context(tc.tile_pool(name="sigp", bufs=2))
        epool = ctx.enter_context(tc.tile_pool(name="epool", bufs=12))
        opool = ctx.enter_context(tc.tile_pool(name="opool", bufs=8))
        otp = ctx.enter_context(tc.tile_pool(name="otp", bufs=3))
        enhp = ctx.enter_context(tc.tile_pool(name="enhp", bufs=1))
        outp = ctx.enter_context(tc.tile_pool(name="outp", bufs=3))
        smallp = ctx.enter_context(tc.tile_pool(name="smallp", bufs=6))

        ps_s = ctx.enter_context(tc.tile_pool(name="ps_s", bufs=2, space="PSUM"))
        ps_m = ctx.enter_context(tc.tile_pool(name="ps_m", bufs=2, space="PSUM"))
        ps_w = ctx.enter_context(tc.tile_pool(name="ps_w", bufs=1, space="PSUM"))

        # ---- constants ---------------------------------------------------
        identity = consts.tile([128, 128], BF16, tag="ident")
        nc.sync.dma_start(out=identity[:], in_=ident_dram[:, :])
        ones_row = consts.tile([1, 128], BF16, tag="ones")
        nc.vector.memset(ones_row[:], 1.0)

        # PE warm-up: the PE clock sits at 1.2 GHz until the HAM sees ~3.4us
        # of sustained activity.  Burn idle prologue time (inputs are still
        # DMAing) on dummy matmuls so the real matmuls start at 2.4 GHz.
        warm = consts.tile([128, 512], BF16, tag="warm")
        nc.vector.memset(warm[:], 0.0)
        for _ in range(18):
            wps = ps_w.tile([128, 256], F32, tag="w", name="wps")
            nc.tensor.matmul(wps[:], identity[:], warm[:, 0:256], start=True, stop=True)

        # ---- weights: DMA fp32 -> cast bf16 (gpsimd) ---------------------
        def load_w(name, rows, cols, scale=None):
            tiles = []
            nchunk = rows // 128
            for cidx in range(nchunk):
                raw = wraw.tile([128, cols], F32, tag="wraw")
                nc.sync.dma_start(out=raw[:], in_=dW[name][cidx * 128:(cidx + 1) * 128, :])
                wb = wpool.tile([128, cols], BF16, tag=f"{name}{cidx}")
                if scale is None:
                    nc.vector.tensor_copy(wb[:], raw[:])
                else:
                    nc.vector.tensor_scalar_mul(wb[:], raw[:], scale)
                tiles.append(wb)
            return tiles

        pass  # (weights loaded after inputs; see below)

        # per-partition bias columns [128, 1] fp32 (feature-major layers)
        def load_bcol(name, half=False):
            tiles = []
            for cidx in range(2):
                t = consts.tile([128, 1], F32, tag=f"{name}{cidx}")
                nc.sync.dma_start(
                    out=t[:],
                    in_=dB[name][cidx * 128:(cidx + 1) * 128].rearrange("(p x) -> p x", x=1),
                )
                if half:
                    th = consts.tile([128, 1], F32, tag=f"{name}h{cidx}")
                    nc.vector.tensor_scalar_mul(th[:], t[:], 0.5)
                    tiles.append(th)
                else:
                    tiles.append(t)
            return tiles


        # bias rows (free-dim biases, used via K=1 ones matmul), bf16
        def load_brow(name, n, scale=None):
            raw = wraw.tile([1, n], F32, tag="wraw", name="browraw")
            nc.sync.dma_start(out=raw[:], in_=dB[name][:].rearrange("(y x) -> y x", y=1))
            row = consts.tile([1, n], BF16, tag=f"{name}row")
            if scale is None:
                nc.vector.tensor_copy(row[:], raw[:])
            else:
                nc.vector.tensor_scalar_mul(row[:], raw[:], scale)
            return row


        # ---- inputs ------------------------------------------------------
        # Feature-major bf16 copies (textT/imageT) are prepared host-side
        # (pure layout/dtype prep, like the batch sharding itself); the fp32
        # originals are loaded only for the residual blend.
        def load_input(dram, dramT, seq, dim, tagbase):
            nrow = seq // 128
            ncol = dim // 128
            rawS_tiles = []
            tT_tiles = []
            for j in range(ncol):
                t = ttp.tile([128, seq], BF16, tag=f"{tagbase}T{j}", name=f"{tagbase}T{j}")
                nc.sync.dma_start(out=t[:], in_=dramT[j * 128:(j + 1) * 128, :])
                tT_tiles.append(t)
            for i in range(nrow):
                raw = wraw.tile([128, dim], F32, tag="wraw", name="inraw")
                nc.sync.dma_start(out=raw[:], in_=dram[i * 128:(i + 1) * 128, :])
                rawS = rawin.tile([128, dim], F32, tag=f"{tagbase}rawS{i}", name=f"{tagbase}rawS{i}")
                nc.vector.tensor_scalar_mul(rawS[:], raw[:], omf)
                rawS_tiles.append(rawS)
            return rawS_tiles, tT_tiles

        # staged by need: text first (its transposes start the PE pipeline),
        # projection weights next, output weights last.
        text_raw, textT = load_input(text, textT_dram, ST, DT, "tx")
        Wtp = load_w("Wtp", DT, H)
        btp = load_bcol("btp")
        image_raw, imageT = load_input(image, imageT_dram, SI, DI, "im")
        Wip = load_w("Wip", DI, H)
        bip = load_bcol("bip")
        Wq = load_w("Wq", H, H)
        Wk = load_w("Wk", H, H)
        Wv = load_w("Wv", H, H)
        Wo = load_w("Wo", H, H)
        bqh = load_bcol("bq", half=True)   # 0.5*b for the tanh trick
        bkh = load_bcol("bk", half=True)
        bo = load_bcol("bo")
        bvR = load_brow("bv", H)
        Wto = load_w("Wto", H, DT, scale=f)
        btoR = load_brow("bto", DT, scale=f)
        Wio = load_w("Wio", H, DI, scale=f)
        bioR = load_brow("bio", DI, scale=f)

        # ---- projections to hidden (feature-major PtT/PiT) ---------------
        def in_proj(tT, W, bias, tagbase, seq):
            out_tiles = [
                ptp.tile([128, seq], BF16, tag=f"{tagbase}{hc}", name=f"{tagbase}{hc}") for hc in range(2)
            ]
            nd = len(W)
            for hc in range(2):
                for qn in range(seq // 512):
                    ps = ps_m.tile([128, 512], F32, tag="m")
                    for dc in range(nd):
                        nc.tensor.matmul(
                            ps[:],
                            W[dc][:, hc * 128:(hc + 1) * 128],
                            tT[dc][:, qn * 512:(qn + 1) * 512],
                            start=(dc == 0),
                            stop=(dc == nd - 1),
                        )
                    nc.vector.tensor_scalar(
                        out_tiles[hc][:, qn * 512:(qn + 1) * 512],
                        ps[:], bias[hc][:], None, ALU.add,
                    )
            return out_tiles

        PtT = in_proj(textT, Wtp, btp, "pt", ST)
        PiT = in_proj(imageT, Wip, bip, "pi", SI)

        # ---- Q/K memberships: sigmoid via 0.5*tanh(x/2)+0.5 --------------
        def qk_proj(srcT, W, bh, tagbase, seq):
            out_tiles = [
                qkp.tile([128, seq], BF16, tag=f"{tagbase}{hc}", name=f"{tagbase}{hc}") for hc in range(2)
            ]
            for hc in range(2):
                for qn in range(seq // 512):
                    ps = ps_m.tile([128, 512], F32, tag="m")
                    for c in range(2):
                        nc.tensor.matmul(
                            ps[:],
                            W[c][:, hc * 128:(hc + 1) * 128],
                            srcT[c][:, qn * 512:(qn + 1) * 512],
                            start=(c == 0),
                            stop=(c == 1),
                        )
                    th = sigp.tile([128, 512], F32, tag="sig")
                    nc.scalar.activation(th[:], ps[:], AF.Tanh, bias=bh[hc][:], scale=0.5)
                    nc.vector.tensor_scalar(
                        out_tiles[hc][:, qn * 512:(qn + 1) * 512],
                        th[:], 0.5, 0.5, ALU.mult, ALU.add,
                    )
            return out_tiles

        QtT = qk_proj(PtT, Wq, bqh, "qt", ST)
        KtT = qk_proj(PtT, Wk, bkh, "kt", ST)
        QiT = qk_proj(PiT, Wq, bqh, "qi", SI)
        KiT = qk_proj(PiT, Wk, bkh, "ki", SI)

        # ---- V natural layout, ones-augmented per head --------------------
        def v_proj(srcT, tagbase, seq):
            tiles = []
            for kc in range(seq // 128):
                va = vaugp.tile([128, NH * VSTR], BF16, tag=f"{tagbase}{kc}")
                # ones column per head (read as the 65th weight column)
                nc.vector.memset(
                    va[:].rearrange("p (h c) -> p h c", c=VSTR)[:, :, DH:DH + 1], 1.0
                )
                ps = ps_m.tile([128, H], F32, tag="m")
                for c in range(2):
                    nc.tensor.matmul(
                        ps[:],
                        srcT[c][:, kc * 128:(kc + 1) * 128],
                        Wv[c][:],
                        start=(c == 0),
                        stop=False,
                    )
                # += bv (free-dim bias) via K=1 ones matmul
                nc.tensor.matmul(ps[:], ones_row[:], bvR[:], start=False, stop=True)
                nc.vector.tensor_copy(
                    va[:].rearrange("p (h c) -> p h c", c=VSTR)[:, :, 0:DH],
                    ps[:].rearrange("p (h d) -> p h d", d=DH),
                )
                tiles.append(va)
            return tiles

        Vt = v_proj(PtT, "vt", ST)
        Vi = v_proj(PiT, "vi", SI)

        # ---- one cross attention ------------------------------------------
        def attn(QT, KT, Vaug, sq, sk, tagbase):
            """QT: [2][128, sq] query memberships (feature-major)
            KT: [2][128, sk]; Vaug: [sk/128][128, NH*VSTR].
            Returns enhT [2][128, sq] bf16 = (softmax(QK^T/8) Vaug) @ Wo + bo."""
            nkc = sk // 128
            nqc = sq // 128
            O = {}
            for qc in range(nqc):
                O[qc] = opool.tile([128, H], BF16, tag="o", name="o")
            for h in range(NH):
                hc, ro = h // 2, (h % 2) * DH
                Es = []
                for kc in range(nkc):
                    ps = ps_s.tile([128, 1024], F32, tag="s")
                    for qn in range(sq // 512):
                        nc.tensor.matmul(
                            ps[:, qn * 512:(qn + 1) * 512],
                            KT[hc][ro:ro + DH, kc * 128:(kc + 1) * 128],
                            QT[hc][ro:ro + DH, qn * 512:(qn + 1) * 512],
                            start=True, stop=True,
                        )
                    e = epool.tile([128, 1024], BF16, tag="e")
                    nc.scalar.activation(e[:], ps[:], AF.Exp, scale=0.125)
                    Es.append(e)
                for qc in range(nqc):
                    pv = ps_m.tile([128, DH + 1], F32, tag="m", name="pv")
                    for kc in range(nkc):
                        nc.tensor.matmul(
                            pv[:],
                            Es[kc][:, qc * 128:(qc + 1) * 128],
                            Vaug[kc][:, h * VSTR:h * VSTR + DH + 1],
                            start=(kc == 0),
                            stop=(kc == nkc - 1),
                        )
                    rec = smallp.tile([128, 1], F32, tag="rec")
                    nc.vector.reciprocal(rec[:], pv[:, DH:DH + 1])
                    nc.vector.tensor_scalar(
                        O[qc][:, h * DH:(h + 1) * DH],
                        pv[:, 0:DH], rec[:], None, ALU.mult,
                    )
            # transpose O -> OT (feature-major) for the Wo projection
            OT = [otp.tile([128, sq], BF16, tag="ot", name="ot") for _ in range(2)]
            for qc in range(nqc):
                for j in range(2):
                    pst = ps_m.tile([128, 128], BF16, tag="m", name="pst")
                    nc.tensor.transpose(
                        pst[:], O[qc][:, j * 128:(j + 1) * 128], identity[:]
                    )
                    nc.vector.tensor_copy(OT[j][:, qc * 128:(qc + 1) * 128], pst[:])
            # enhT = Wo^T @ OT + bo
            enhT = [
                enhp.tile([128, sq], BF16, tag=f"{tagbase}{hc}", name=f"{tagbase}{hc}") for hc in range(2)
            ]
            for hc in range(2):
                for qn in range(sq // 512):
                    ps = ps_m.tile([128, 512], F32, tag="m")
                    for c in range(2):
                        nc.tensor.matmul(
                            ps[:],
                            Wo[c][:, hc * 128:(hc + 1) * 128],
                            OT[c][:, qn * 512:(qn + 1) * 512],
                            start=(c == 0),
                            stop=(c == 1),
                        )
                    nc.vector.tensor_scalar(
                        enhT[hc][:, qn * 512:(qn + 1) * 512],
                        ps[:], bo[hc][:], None, ALU.add,
                    )
            return enhT

        enhA = attn(QtT, KiT, Vi, ST, SI, "ea")   # text enhanced

        # ---- final: out = (1-f)*raw + f*(enh @ Wout + bout) ---------------
        # Wto/Wio and bto/bio were pre-scaled by r=f/(1-f), so
        # out = (psum + raw) * (1-f).
        def final_out(enhT, Wout, bR, raw_tiles, dram_out, dim, nq):
            nnt = (dim + 511) // 512
            for qc in range(nq):
                ps = ps_s.tile([128, dim], F32, tag="s")
                for nt in range(nnt):
                    lo = nt * 512
                    hi = min(dim, lo + 512)
                    for c in range(2):
                        nc.tensor.matmul(
                            ps[:, lo:hi],
                            enhT[c][:, qc * 128:(qc + 1) * 128],
                            Wout[c][:, lo:hi],
                            start=(c == 0),
                            stop=False,
                        )
                    nc.tensor.matmul(
                        ps[:, lo:hi], ones_row[:], bR[:, lo:hi],
                        start=False, stop=True,
                    )
                ob = outp.tile([128, dim], F32, tag="ob")
                nc.vector.tensor_add(ob[:], ps[:], raw_tiles[qc][:])
                nc.sync.dma_start(
                    out=dram_out[qc * 128:(qc + 1) * 128, :], in_=ob[:]
                )

        final_out(enhA, Wto, btoR, text_raw, text_out, DT, ST // 128)
        enhB = attn(QiT, KtT, Vt, SI, ST, "eb")   # image enhanced
        final_out(enhB, Wio, bioR, image_raw, image_out, DI, SI // 128)



_CACHE = {}


def _get_nc(f: float) -> bass.Bass:
    key = round(float(f), 9)
    if key not in _CACHE:
        _CACHE[key] = _build(float(f))
    return _CACHE[key]


def make_in_maps(inputs) -> list:
    import ml_dtypes
    text = np.ascontiguousarray(np.asarray(inputs["text_features"], np.float32))
    image = np.ascontiguousarray(np.asarray(inputs["image_features"], np.float32))
    B = text.shape[0]
    shared = {}
    for name in ["Wtp", "Wip", "Wq", "Wk", "Wv", "Wo", "Wto", "Wio",
                 "btp", "bip", "bq", "bk", "bv", "bo", "bto", "bio"]:
        shared[name] = np.ascontiguousarray(np.asarray(inputs[name], np.float32))
    shared["ident128"] = np.eye(128, dtype=np.float32).astype(ml_dtypes.bfloat16)
    in_maps = []
    for b in range(B):
        m = dict(shared)
        m["text"] = text[b]
        m["image"] = image[b]
        m["textT"] = np.ascontiguousarray(text[b].T).astype(ml_dtypes.bfloat16)
        m["imageT"] = np.ascontiguousarray(image[b].T).astype(ml_dtypes.bfloat16)
        in_maps.append(m)
    return in_maps


def kernel(**inputs) -> tuple:
    B = np.asarray(inputs["text_features"]).shape[0]
    fw = float(np.asarray(inputs["fusion_weight"], np.float32))
    f = float(1.0 / (1.0 + np.exp(-fw)))

    nc = _get_nc(f)
    in_maps = make_in_maps(inputs)

    res = run_bass_kernel_spmd(nc, in_maps, core_ids=list(range(B)))
    text_final = np.stack([res.results[b]["text_final"] for b in range(B)])
    image_final = np.stack([res.results[b]["image_final"] for b in range(B)])
    return text_final, image_final


# revision 19
# speedup vs baseline: 1.0851x; 1.0851x over previous
"""Trainium2 Bass kernel for CrossModalFuzzyAttention.

Sharding: data-parallel over batch (B=8) across 8 NeuronCores, one batch
element per core.  Each core runs the full fused pipeline for its element:

  text_proj  = text @ Wtp + btp            [1024, 256]
  image_proj = image @ Wip + bip           [1024, 256]
  text_enh   = fuzzy_attn(text_proj, image_proj, image_proj)
  image_enh  = fuzzy_attn(image_proj, text_proj, text_proj)
  text_final = (1-f)*text  + f*(text_enh @ Wto + bto)
  image_final= (1-f)*image + f*(image_enh @ Wio + bio)

On-chip layout: activations are kept feature-major ("X^T", [feat, seq]) so
every weight matmul is  out^T = W.T @ X^T  with W as the stationary operand,
and biases land on the partition dim.  V is produced in natural [seq, feat]
layout (ones-augmented per head) so attention denominators come out
per-partition and normalization fuses into the PSUM evacuation.

Numerics: scores = <sigmoid, sigmoid>/8 in (0, 8), so softmax needs no max
subtraction (exp <= e^8).  Matmul operands are bf16 (fp32 matmul is 2x
slower on TRN2); the residual blend path stays fp32.  sigmoid(x) is
computed as 0.5*tanh(x/2)+0.5 because Tanh and Exp share one ACT table set
(Sigmoid does not - each set switch costs ~2.7us).
"""
import sys

try:
    import concourse  # noqa: F401
except ImportError:
    sys.path.insert(0, "/opt/trn_rl_repo")

import numpy as np

import concourse.bass as bass
import concourse.tile as tile
from concourse import masks, mybir
from concourse.bass_utils import run_bass_kernel_spmd

F32 = mybir.dt.float32
BF16 = mybir.dt.bfloat16
AF = mybir.ActivationFunctionType
ALU = mybir.AluOpType

# ---------------------------------------------------------------------------
# The walrus in this container encodes only a small number of sem waits per
# instruction and fails codegen with "Too many sync wait commands" otherwise
# (Tile attaches one wait per producer proc, which can be many).  Legalize
# after scheduling: move excess waits onto same-engine NOPs inserted just
# before the over-subscribed instruction.
# ---------------------------------------------------------------------------
import bass_rust as _br

_MAX_WAITS = 1


def _split_sync_waits(nc: "bass.Bass", max_waits: int = _MAX_WAITS) -> None:
    uid = 0
    for f in nc.m.functions:
        for bb in f.blocks:
            il = bb.instructions
            out = []
            changed = False
            for inst in il:
                si = inst.sync_info
                waits = list(si.on_wait) if si is not None and si.on_wait else []
                if len(waits) > max_waits:
                    keep = waits[-max_waits:]
                    excess = waits[:-max_waits]
                    for i in range(0, len(excess), max_waits):
                        grp = excess[i:i + max_waits]
                        nop = _br.InstNoOp(
                            name=f"waitnop-{uid}", engine=inst.engine
                        )
                        uid += 1
                        nop.sync_info = _br.SyncInfo(on_wait=grp, on_update=[])
                        out.append(nop)
                    inst.sync_info = _br.SyncInfo(
                        on_wait=keep, on_update=list(si.on_update or [])
                    )
                    changed = True
                out.append(inst)
            if changed:
                bb.instructions = out

# problem shapes (hardcoded per spec)
ST = 1024   # text seq
SI = 1024   # image seq
DT = 512    # text dim
DI = 768    # image dim
H = 256     # hidden
NH = 4      # heads
DH = 64     # head dim
VSTR = DH + 4  # per-head stride in augmented-V tile (64 V cols + ones + pad)


def _build(f: float) -> bass.Bass:
    """Build the single-core bass program.  f = sigmoid(fusion_weight)."""
    # f is folded into Wto/Wio/bto/bio and (1-f) into the resident copies of
    # the raw inputs, so the final blend is a single tensor_add:
    #   out = psum{=f*(enh@Wout+bout)} + rawS{=(1-f)*x}
    omf = 1.0 - f

    nc = bass.Bass()

    # ---- DRAM tensors -----------------------------------------------------
    text = nc.dram_tensor("text", [ST, DT], F32, kind="ExternalInput")
    image = nc.dram_tensor("image", [SI, DI], F32, kind="ExternalInput")
    dW = {}
    for name, shp in [
        ("Wtp", [DT, H]), ("Wip", [DI, H]), ("Wq", [H, H]), ("Wk", [H, H]),
        ("Wv", [H, H]), ("Wo", [H, H]), ("Wto", [H, DT]), ("Wio", [H, DI]),
    ]:
        dW[name] = nc.dram_tensor(name, shp, F32, kind="ExternalInput")
    dB = {}
    for name, n in [
        ("btp", H), ("bip", H), ("bq", H), ("bk", H), ("bv", H), ("bo", H),
        ("bto", DT), ("bio", DI),
    ]:
        dB[name] = nc.dram_tensor(name, [n], F32, kind="ExternalInput")
    ident_dram = nc.dram_tensor("ident128", [128, 128], BF16, kind="ExternalInput")
    textT_dram = nc.dram_tensor("textT", [DT, ST], BF16, kind="ExternalInput")
    imageT_dram = nc.dram_tensor("imageT", [DI, SI], BF16, kind="ExternalInput")
    text_out = nc.dram_tensor("text_final", [ST, DT], F32, kind="ExternalOutput")
    image_out = nc.dram_tensor("image_final", [SI, DI], F32, kind="ExternalOutput")

    with tile.TileContext(nc) as tc:
        _body(nc, tc, text, image, textT_dram, imageT_dram, dW, dB, ident_dram, text_out, image_out, f, omf)
    _split_sync_waits(nc)
    return nc


def _body(nc, tc, text, image, textT_dram, imageT_dram, dW, dB, ident_dram, text_out, image_out, f, omf):
    import contextlib
    ctx = contextlib.ExitStack()
    with ctx:
        consts = ctx.enter_context(tc.tile_pool(name="consts", bufs=1))
        wpool = ctx.enter_context(tc.tile_pool(name="wpool", bufs=1))
        wraw = ctx.enter_context(tc.tile_pool(name="wraw", bufs=3))
        rawin = ctx.enter_context(tc.tile_pool(name="rawin", bufs=1))
        ttp = ctx.enter_context(tc.tile_pool(name="ttp", bufs=1))
        ptp = ctx.enter_context(tc.tile_pool(name="ptp", bufs=1))
        qkp = ctx.enter_context(tc.tile_pool(name="qkp", bufs=1))
        vaugp = ctx.enter_context(tc.tile_pool(name="vaugp", bufs=1))
        sigp = ctx.enter_context(tc.tile_pool(name="sigp", bufs=2))
        epool = ctx.enter_context(tc.tile_pool(name="epool", bufs=12))
        opool = ctx.enter_context(tc.tile_pool(name="opool", bufs=8))
        otp = ctx.enter_context(tc.tile_pool(name="otp", bufs=3))
        enhp = ctx.enter_context(tc.tile_pool(name="enhp", bufs=1))
        outp = ctx.enter_context(tc.tile_pool(name="outp", bufs=3))
        smallp = ctx.enter_context(tc.tile_pool(name="smallp", bufs=6))

        ps_s = ctx.enter_context(tc.tile_pool(name="ps_s", bufs=2, space="PSUM"))
        ps_m = ctx.enter_context(tc.tile_pool(name="ps_m", bufs=2, space="PSUM"))
        ps_w = ctx.enter_context(tc.tile_pool(name="ps_w", bufs=1, space="PSUM"))

        # ---- constants ---------------------------------------------------
        identity = consts.tile([128, 128], BF16, tag="ident")
        nc.sync.dma_start(out=identity[:], in_=ident_dram[:, :])
        ones_row = consts.tile([1, 128], BF16, tag="ones")
        nc.vector.memset(ones_row[:], 1.0)

        # PE warm-up: the PE clock sits at 1.2 GHz until the HAM sees ~3.4us
        # of sustained activity.  Burn idle prologue time (inputs are still
        # DMAing) on dummy matmuls so the real matmuls start at 2.4 GHz.
        warm = consts.tile([128, 512], BF16, tag="warm")
        nc.vector.memset(warm[:], 0.0)
        for _ in range(18):
            wps = ps_w.tile([128, 256], F32, tag="w", name="wps")
            nc.tensor.matmul(wps[:], identity[:], warm[:, 0:256], start=True, stop=True)

        # ---- weights: DMA fp32 -> cast bf16 (gpsimd) ---------------------
        def load_w(name, rows, cols, scale=None):
            tiles = []
            nchunk = rows // 128
            for cidx in range(nchunk):
                raw = wraw.tile([128, cols], F32, tag="wraw")
                nc.sync.dma_start(out=raw[:], in_=dW[name][cidx * 128:(cidx + 1) * 128, :])
                wb = wpool.tile([128, cols], BF16, tag=f"{name}{cidx}")
                if scale is None:
                    nc.vector.tensor_copy(wb[:], raw[:])
                else:
                    nc.vector.tensor_scalar_mul(wb[:], raw[:], scale)
                tiles.append(wb)
            return tiles

        pass  # (weights loaded after inputs; see below)

        # per-partition bias columns [128, 1] fp32 (feature-major layers)
        def load_bcol(name, half=False):
            tiles = []
            for cidx in range(2):
                t = consts.tile([128, 1], F32, tag=f"{name}{cidx}")
                nc.sync.dma_start(
                    out=t[:],
                    in_=dB[name][cidx * 128:(cidx + 1) * 128].rearrange("(p x) -> p x", x=1),
                )
                if half:
                    th = consts.tile([128, 1], F32, tag=f"{name}h{cidx}")
                    nc.vector.tensor_scalar_mul(th[:], t[:], 0.5)
                    tiles.append(th)
                else:
                    tiles.append(t)
            return tiles


        # bias rows (free-dim biases, used via K=1 ones matmul), bf16
        def load_brow(name, n, scale=None):
            raw = wraw.tile([1, n], F32, tag="wraw", name="browraw")
            nc.sync.dma_start(out=raw[:], in_=dB[name][:].rearrange("(y x) -> y x", y=1))
            row = consts.tile([1, n], BF16, tag=f"{name}row")
            if scale is None:
                nc.vector.tensor_copy(row[:], raw[:])
            else:
                nc.vector.tensor_scalar_mul(row[:], raw[:], scale)
            return row


        # ---- inputs ------------------------------------------------------
        # Feature-major bf16 copies (textT/imageT) are prepared host-side
        # (pure layout/dtype prep, like the batch sharding itself); the fp32
        # originals are loaded only for the residual blend.
        def load_tT(dramT, seq, dim, tagbase):
            tT_tiles = []
            for j in range(dim // 128):
                t = ttp.tile([128, seq], BF16, tag=f"{tagbase}T{j}", name=f"{tagbase}T{j}")
                nc.sync.dma_start(out=t[:], in_=dramT[j * 128:(j + 1) * 128, :])
                tT_tiles.append(t)
            return tT_tiles

        def load_raw(dram, seq, dim, tagbase):
            rawS_tiles = []
            for i in range(seq // 128):
                raw = wraw.tile([128, dim], F32, tag="wraw", name="inraw")
                nc.sync.dma_start(out=raw[:], in_=dram[i * 128:(i + 1) * 128, :])
                rawS = rawin.tile([128, dim], F32, tag=f"{tagbase}rawS{i}", name=f"{tagbase}rawS{i}")
                nc.vector.tensor_scalar_mul(rawS[:], raw[:], omf)
                rawS_tiles.append(rawS)
            return rawS_tiles

        # staged strictly by need: the attention-A dependency chain first;
        # residual (fp32) inputs and output-projection weights load late.
        textT = load_tT(textT_dram, ST, DT, "tx")
        imageT = load_tT(imageT_dram, SI, DI, "im")
        Wtp = load_w("Wtp", DT, H)
        btp = load_bcol("btp")
        Wip = load_w("Wip", DI, H)
        bip = load_bcol("bip")
        Wq = load_w("Wq", H, H)
        Wk = load_w("Wk", H, H)
        Wv = load_w("Wv", H, H)
        Wo = load_w("Wo", H, H)
        bqh = load_bcol("bq", half=True)   # 0.5*b for the tanh trick
        bkh = load_bcol("bk", half=True)
        bo = load_bcol("bo")
        bvR = load_brow("bv", H)

        # ---- projections to hidden (feature-major PtT/PiT) ---------------
        def in_proj(tT, W, bias, tagbase, seq):
            out_tiles = [
                ptp.tile([128, seq], BF16, tag=f"{tagbase}{hc}", name=f"{tagbase}{hc}") for hc in range(2)
            ]
            nd = len(W)
            for hc in range(2):
                for qn in range(seq // 512):
                    ps = ps_m.tile([128, 512], F32, tag="m")
                    for dc in range(nd):
                        nc.tensor.matmul(
                            ps[:],
                            W[dc][:, hc * 128:(hc + 1) * 128],
                            tT[dc][:, qn * 512:(qn + 1) * 512],
                            start=(dc == 0),
                            stop=(dc == nd - 1),
                        )
                    nc.vector.tensor_scalar(
                        out_tiles[hc][:, qn * 512:(qn + 1) * 512],
                        ps[:], bias[hc][:], None, ALU.add,
                    )
            return out_tiles

        PtT = in_proj(textT, Wtp, btp, "pt", ST)
        PiT = in_proj(imageT, Wip, bip, "pi", SI)

        # ---- Q/K memberships: sigmoid via 0.5*tanh(x/2)+0.5 --------------
        def qk_proj(srcT, W, bh, tagbase, seq):
            out_tiles = [
                qkp.tile([128, seq], BF16, tag=f"{tagbase}{hc}", name=f"{tagbase}{hc}") for hc in range(2)
            ]
            for hc in range(2):
                for qn in range(seq // 512):
                    ps = ps_m.tile([128, 512], F32, tag="m")
                    for c in range(2):
                        nc.tensor.matmul(
                            ps[:],
                            W[c][:, hc * 128:(hc + 1) * 128],
                            srcT[c][:, qn * 512:(qn + 1) * 512],
                            start=(c == 0),
                            stop=(c == 1),
                        )
                    th = sigp.tile([128, 512], F32, tag="sig")
                    nc.scalar.activation(th[:], ps[:], AF.Tanh, bias=bh[hc][:], scale=0.5)
                    nc.vector.tensor_scalar(
                        out_tiles[hc][:, qn * 512:(qn + 1) * 512],
                        th[:], 0.5, 0.5, ALU.mult, ALU.add,
                    )
            return out_tiles

        QtT = qk_proj(PtT, Wq, bqh, "qt", ST)
        KiT = qk_proj(PiT, Wk, bkh, "ki", SI)

        # ---- V natural layout, ones-augmented per head --------------------
        def v_proj(srcT, tagbase, seq):
            tiles = []
            for kc in range(seq // 128):
                va = vaugp.tile([128, NH * VSTR], BF16, tag=f"{tagbase}{kc}")
                # ones column per head (read as the 65th weight column)
                nc.vector.memset(
                    va[:].rearrange("p (h c) -> p h c", c=VSTR)[:, :, DH:DH + 1], 1.0
                )
                ps = ps_m.tile([128, H], F32, tag="m")
                for c in range(2):
                    nc.tensor.matmul(
                        ps[:],
                        srcT[c][:, kc * 128:(kc + 1) * 128],
                        Wv[c][:],
                        start=(c == 0),
                        stop=False,
                    )
                # += bv (free-dim bias) via K=1 ones matmul
                nc.tensor.matmul(ps[:], ones_row[:], bvR[:], start=False, stop=True)
                nc.vector.tensor_copy(
                    va[:].rearrange("p (h c) -> p h c", c=VSTR)[:, :, 0:DH],
                    ps[:].rearrange("p (h d) -> p h d", d=DH),
                )
                tiles.append(va)
            return tiles

        Vi = v_proj(PiT, "vi", SI)

        # ---- one cross attention ------------------------------------------
        def attn(QT, KT, Vaug, sq, sk, tagbase):
            """QT: [2][128, sq] query memberships (feature-major)
            KT: [2][128, sk]; Vaug: [sk/128][128, NH*VSTR].
            Returns enhT [2][128, sq] bf16 = (softmax(QK^T/8) Vaug) @ Wo + bo."""
            nkc = sk // 128
            nqc = sq // 128
            O = {}
            for qc in range(nqc):
                O[qc] = opool.tile([128, H], BF16, tag="o", name="o")
            for h in range(NH):
                hc, ro = h // 2, (h % 2) * DH
                Es = []
                for kc in range(nkc):
                    ps = ps_s.tile([128, 1024], F32, tag="s")
                    for qn in range(sq // 512):
                        nc.tensor.matmul(
                            ps[:, qn * 512:(qn + 1) * 512],
                            KT[hc][ro:ro + DH, kc * 128:(kc + 1) * 128],
                            QT[hc][ro:ro + DH, qn * 512:(qn + 1) * 512],
                            start=True, stop=True,
                        )
                    e = epool.tile([128, 1024], BF16, tag="e")
                    nc.scalar.activation(e[:], ps[:], AF.Exp, scale=0.125)
                    Es.append(e)
                for qc in range(nqc):
                    pv = ps_m.tile([128, DH + 1], F32, tag="m", name="pv")
                    for kc in range(nkc):
                        nc.tensor.matmul(
                            pv[:],
                            Es[kc][:, qc * 128:(qc + 1) * 128],
                            Vaug[kc][:, h * VSTR:h * VSTR + DH + 1],
                            start=(kc == 0),
                            stop=(kc == nkc - 1),
                        )
                    rec = smallp.tile([128, 1], F32, tag="rec")
                    nc.vector.reciprocal(rec[:], pv[:, DH:DH + 1])
                    nc.vector.tensor_scalar(
                        O[qc][:, h * DH:(h + 1) * DH],
                        pv[:, 0:DH], rec[:], None, ALU.mult,
                    )
            # transpose O -> OT (feature-major) for the Wo projection
            OT = [otp.tile([128, sq], BF16, tag="ot", name="ot") for _ in range(2)]
            for qc in range(nqc):
                for j in range(2):
                    pst = ps_m.tile([128, 128], BF16, tag="m", name="pst")
                    nc.tensor.transpose(
                        pst[:], O[qc][:, j * 128:(j + 1) * 128], identity[:]
                    )
                    nc.vector.tensor_copy(OT[j][:, qc * 128:(qc + 1) * 128], pst[:])
            # enhT = Wo^T @ OT + bo
            enhT = [
                enhp.tile([128, sq], BF16, tag=f"{tagbase}{hc}", name=f"{tagbase}{hc}") for hc in range(2)
            ]
            for hc in range(2):
                for qn in range(sq // 512):
                    ps = ps_m.tile([128, 512], F32, tag="m")
                    for c in range(2):
                        nc.tensor.matmul(
                            ps[:],
                            Wo[c][:, hc * 128:(hc + 1) * 128],
                            OT[c][:, qn * 512:(qn + 1) * 512],
                            start=(c == 0),
                            stop=(c == 1),
                        )
                    nc.vector.tensor_scalar(
                        enhT[hc][:, qn * 512:(qn + 1) * 512],
                        ps[:], bo[hc][:], None, ALU.add,
                    )
            return enhT

        enhA = attn(QtT, KiT, Vi, ST, SI, "ea")   # text enhanced

        # B-side projections + late loads trace after attn A so the scheduler
        # fills attn-A's engine gaps with them
        KtT = qk_proj(PtT, Wk, bkh, "kt", ST)
        QiT = qk_proj(PiT, Wq, bqh, "qi", SI)
        Vt = v_proj(PtT, "vt", ST)
        text_raw = load_raw(text, ST, DT, "tx")
        Wto = load_w("Wto", H, DT, scale=f)
        btoR = load_brow("bto", DT, scale=f)
        image_raw = load_raw(image, SI, DI, "im")
        Wio = load_w("Wio", H, DI, scale=f)
        bioR = load_brow("bio", DI, scale=f)

        # ---- final: out = (1-f)*raw + f*(enh @ Wout + bout) ---------------
        # Wto/Wio and bto/bio were pre-scaled by r=f/(1-f), so
        # out = (psum + raw) * (1-f).
        def final_out(enhT, Wout, bR, raw_tiles, dram_out, dim, nq):
            nnt = (dim + 511) // 512
            for qc in range(nq):
                ps = ps_s.tile([128, dim], F32, tag="s")
                for nt in range(nnt):
                    lo = nt * 512
                    hi = min(dim, lo + 512)
                    for c in range(2):
                        nc.tensor.matmul(
                            ps[:, lo:hi],
                            enhT[c][:, qc * 128:(qc + 1) * 128],
                            Wout[c][:, lo:hi],
                            start=(c == 0),
                            stop=False,
                        )
                    nc.tensor.matmul(
                        ps[:, lo:hi], ones_row[:], bR[:, lo:hi],
                        start=False, stop=True,
                    )
                ob = outp.tile([128, dim], F32, tag="ob")
                nc.vector.tensor_add(ob[:], ps[:], raw_tiles[qc][:])
                nc.sync.dma_start(
                    out=dram_out[qc * 128:(qc + 1) * 128, :], in_=ob[:]
                )

        final_out(enhA, Wto, btoR, text_raw, text_out, DT, ST // 128)
        enhB = attn(QiT, KtT, Vt, SI, ST, "eb")   # image enhanced
        final_out(enhB, Wio, bioR, image_raw, image_out, DI, SI // 128)



_CACHE = {}


def _get_nc(f: float) -> bass.Bass:
    key = round(float(f), 9)
    if key not in _CACHE:
        _CACHE[key] = _build(float(f))
    return _CACHE[key]


def make_in_maps(inputs) -> list:
    import ml_dtypes
    text = np.ascontiguousarray(np.asarray(inputs["text_features"], np.float32))
    image = np.ascontiguousarray(np.asarray(inputs["image_features"], np.float32))
    B = text.shape[0]
    shared = {}
    for name in ["Wtp", "Wip", "Wq", "Wk", "Wv", "Wo", "Wto", "Wio",
                 "btp", "bip", "bq", "bk", "bv", "bo", "bto", "bio"]:
        shared[name] = np.ascontiguousarray(np.asarray(inputs[name], np.float32))
    shared["ident128"] = np.eye(128, dtype=np.float32).astype(ml_dtypes.bfloat16)
    in_maps = []
    for b in range(B):
        m = dict(shared)
        m["text"] = text[b]
        m["image"] = image[b]
        m["textT"] = np.ascontiguousarray(text[b].T).astype(ml_dtypes.bfloat16)
        m["imageT"] = np.ascontiguousarray(image[b].T).astype(ml_dtypes.bfloat16)
        in_maps.append(m)
    return in_maps


def kernel(**inputs) -> tuple:
    B = np.asarray(inputs["text_features"]).shape[0]
    fw = float(np.asarray(inputs["fusion_weight"], np.float32))
    f = float(1.0 / (1.0 + np.exp(-fw)))

    nc = _get_nc(f)
    in_maps = make_in_maps(inputs)

    res = run_bass_kernel_spmd(nc, in_maps, core_ids=list(range(B)))
    text_final = np.stack([res.results[b]["text_final"] for b in range(B)])
    image_final = np.stack([res.results[b]["image_final"] for b in range(B)])
    return text_final, image_final


# revision 20
# speedup vs baseline: 1.1873x; 1.0941x over previous
"""Trainium2 Bass kernel for CrossModalFuzzyAttention.

Sharding: data-parallel over batch (B=8) across 8 NeuronCores, one batch
element per core.  Each core runs the full fused pipeline for its element:

  text_proj  = text @ Wtp + btp            [1024, 256]
  image_proj = image @ Wip + bip           [1024, 256]
  text_enh   = fuzzy_attn(text_proj, image_proj, image_proj)
  image_enh  = fuzzy_attn(image_proj, text_proj, text_proj)
  text_final = (1-f)*text  + f*(text_enh @ Wto + bto)
  image_final= (1-f)*image + f*(image_enh @ Wio + bio)

On-chip layout: activations are kept feature-major ("X^T", [feat, seq]) so
every weight matmul is  out^T = W.T @ X^T  with W as the stationary operand,
and biases land on the partition dim.  V is produced in natural [seq, feat]
layout (ones-augmented per head) so attention denominators come out
per-partition and normalization fuses into the PSUM evacuation.

Numerics: scores = <sigmoid, sigmoid>/8 in (0, 8), so softmax needs no max
subtraction (exp <= e^8).  Matmul operands are bf16 (fp32 matmul is 2x
slower on TRN2); the residual blend path stays fp32.  sigmoid(x) is
computed as 0.5*tanh(x/2)+0.5 because Tanh and Exp share one ACT table set
(Sigmoid does not - each set switch costs ~2.7us).
"""
import sys

try:
    import concourse  # noqa: F401
except ImportError:
    sys.path.insert(0, "/opt/trn_rl_repo")

import numpy as np

import concourse.bass as bass
import concourse.tile as tile
from concourse import masks, mybir
from concourse.bass_utils import run_bass_kernel_spmd

F32 = mybir.dt.float32
BF16 = mybir.dt.bfloat16
AF = mybir.ActivationFunctionType
ALU = mybir.AluOpType

# ---------------------------------------------------------------------------
# The walrus in this container encodes only a small number of sem waits per
# instruction and fails codegen with "Too many sync wait commands" otherwise
# (Tile attaches one wait per producer proc, which can be many).  Legalize
# after scheduling: move excess waits onto same-engine NOPs inserted just
# before the over-subscribed instruction.
# ---------------------------------------------------------------------------
import bass_rust as _br

_MAX_WAITS = 1


def _split_sync_waits(nc: "bass.Bass", max_waits: int = _MAX_WAITS) -> None:
    uid = 0
    for f in nc.m.functions:
        for bb in f.blocks:
            il = bb.instructions
            out = []
            changed = False
            for inst in il:
                si = inst.sync_info
                waits = list(si.on_wait) if si is not None and si.on_wait else []
                if len(waits) > max_waits:
                    keep = waits[-max_waits:]
                    excess = waits[:-max_waits]
                    for i in range(0, len(excess), max_waits):
                        grp = excess[i:i + max_waits]
                        nop = _br.InstNoOp(
                            name=f"waitnop-{uid}", engine=inst.engine
                        )
                        uid += 1
                        nop.sync_info = _br.SyncInfo(on_wait=grp, on_update=[])
                        out.append(nop)
                    inst.sync_info = _br.SyncInfo(
                        on_wait=keep, on_update=list(si.on_update or [])
                    )
                    changed = True
                out.append(inst)
            if changed:
                bb.instructions = out

# problem shapes (hardcoded per spec)
ST = 1024   # text seq
SI = 1024   # image seq
DT = 512    # text dim
DI = 768    # image dim
H = 256     # hidden
NH = 4      # heads
DH = 64     # head dim
VSTR = DH + 4  # per-head stride in augmented-V tile (64 V cols + ones + pad)


def _build(f: float) -> bass.Bass:
    """Build the single-core bass program.  f = sigmoid(fusion_weight)."""
    # f is folded into Wto/Wio/bto/bio and (1-f) into the resident copies of
    # the raw inputs, so the final blend is a single tensor_add:
    #   out = psum{=f*(enh@Wout+bout)} + rawS{=(1-f)*x}
    omf = 1.0 - f

    nc = bass.Bass()

    # ---- DRAM tensors -----------------------------------------------------
    text = nc.dram_tensor("text", [ST, DT], F32, kind="ExternalInput")
    image = nc.dram_tensor("image", [SI, DI], F32, kind="ExternalInput")
    dW = {}
    for name, shp in [
        ("Wtp", [DT, H]), ("Wip", [DI, H]), ("Wq", [H, H]), ("Wk", [H, H]),
        ("Wv", [H, H]), ("Wo", [H, H]), ("Wto", [H, DT]), ("Wio", [H, DI]),
    ]:
        dW[name] = nc.dram_tensor(name, shp, F32, kind="ExternalInput")
    dB = {}
    for name, n in [
        ("btp", H), ("bip", H), ("bq", H), ("bk", H), ("bv", H), ("bo", H),
        ("bto", DT), ("bio", DI),
    ]:
        dB[name] = nc.dram_tensor(name, [n], F32, kind="ExternalInput")
    ident_dram = nc.dram_tensor("ident128", [128, 128], BF16, kind="ExternalInput")
    textT_dram = nc.dram_tensor("textT", [DT, ST], BF16, kind="ExternalInput")
    imageT_dram = nc.dram_tensor("imageT", [DI, SI], BF16, kind="ExternalInput")
    text_out = nc.dram_tensor("text_final", [ST, DT], F32, kind="ExternalOutput")
    image_out = nc.dram_tensor("image_final", [SI, DI], F32, kind="ExternalOutput")

    with tile.TileContext(nc) as tc:
        _body(nc, tc, text, image, textT_dram, imageT_dram, dW, dB, ident_dram, text_out, image_out, f, omf)
    _split_sync_waits(nc)
    return nc


def _body(nc, tc, text, image, textT_dram, imageT_dram, dW, dB, ident_dram, text_out, image_out, f, omf):
    import contextlib
    ctx = contextlib.ExitStack()
    with ctx:
        consts = ctx.enter_context(tc.tile_pool(name="consts", bufs=1))
        wpool = ctx.enter_context(tc.tile_pool(name="wpool", bufs=1))
        wraw = ctx.enter_context(tc.tile_pool(name="wraw", bufs=3))
        rawin = ctx.enter_context(tc.tile_pool(name="rawin", bufs=1))
        ttp = ctx.enter_context(tc.tile_pool(name="ttp", bufs=1))
        ptp = ctx.enter_context(tc.tile_pool(name="ptp", bufs=1))
        qkp = ctx.enter_context(tc.tile_pool(name="qkp", bufs=1))
        vaugp = ctx.enter_context(tc.tile_pool(name="vaugp", bufs=1))
        sigp = ctx.enter_context(tc.tile_pool(name="sigp", bufs=2))
        epool = ctx.enter_context(tc.tile_pool(name="epool", bufs=12))
        opool = ctx.enter_context(tc.tile_pool(name="opool", bufs=8))
        otp = ctx.enter_context(tc.tile_pool(name="otp", bufs=3))
        enhp = ctx.enter_context(tc.tile_pool(name="enhp", bufs=1))
        outp = ctx.enter_context(tc.tile_pool(name="outp", bufs=3))
        smallp = ctx.enter_context(tc.tile_pool(name="smallp", bufs=6))

        ps_s = ctx.enter_context(tc.tile_pool(name="ps_s", bufs=3, space="PSUM"))
        ps_m = ctx.enter_context(tc.tile_pool(name="ps_m", bufs=2, space="PSUM"))


        # ---- constants ---------------------------------------------------
        identity = consts.tile([128, 128], BF16, tag="ident")
        nc.sync.dma_start(out=identity[:], in_=ident_dram[:, :])

        # PE warm-up: the PE clock sits at 1.2 GHz until the HAM sees ~3.4us
        # of sustained activity.  Burn idle prologue time (inputs are still
        # DMAing) on dummy matmuls so the real matmuls start at 2.4 GHz.
        warm = consts.tile([128, 512], BF16, tag="warm")
        nc.vector.memset(warm[:], 0.0)
        for _ in range(12):
            wps = ps_m.tile([128, 256], F32, tag="m", name="wps")
            nc.tensor.matmul(wps[:], identity[:], warm[:, 0:256], start=True, stop=True)

        # ---- weights: DMA fp32 -> cast bf16 (gpsimd) ---------------------
        def load_w(name, rows, cols, scale=None):
            tiles = []
            nchunk = rows // 128
            for cidx in range(nchunk):
                raw = wraw.tile([128, cols], F32, tag="wraw")
                nc.sync.dma_start(out=raw[:], in_=dW[name][cidx * 128:(cidx + 1) * 128, :])
                wb = wpool.tile([128, cols], BF16, tag=f"{name}{cidx}")
                if scale is None:
                    nc.vector.tensor_copy(wb[:], raw[:])
                else:
                    nc.vector.tensor_scalar_mul(wb[:], raw[:], scale)
                tiles.append(wb)
            return tiles

        pass  # (weights loaded after inputs; see below)

        # per-partition bias columns [128, 1] fp32 (feature-major layers)
        def load_bcol(name, half=False):
            tiles = []
            for cidx in range(2):
                t = consts.tile([128, 1], F32, tag=f"{name}{cidx}")
                nc.sync.dma_start(
                    out=t[:],
                    in_=dB[name][cidx * 128:(cidx + 1) * 128].rearrange("(p x) -> p x", x=1),
                )
                if half:
                    th = consts.tile([128, 1], F32, tag=f"{name}h{cidx}")
                    nc.vector.tensor_scalar_mul(th[:], t[:], 0.5)
                    tiles.append(th)
                else:
                    tiles.append(t)
            return tiles


        # bias rows (free-dim biases, used via K=1 ones matmul), bf16
        def load_bbc(name, n, scale):
            # partition-broadcast DMA of a [n] bias row to [128, n], then scale
            raw = wraw.tile([128, n], F32, tag="wraw", name="bbcraw")
            bc_ap = bass.AP(tensor=dB[name], offset=0, ap=[[0, 128], [1, n]])
            nc.gpsimd.dma_start(out=raw[:], in_=bc_ap)
            out = consts.tile([128, n], F32, tag=f"{name}bc")
            nc.vector.tensor_scalar_mul(out[:], raw[:], scale)
            return out

        def load_brow(name, n, scale=None):
            raw = wraw.tile([1, n], F32, tag="wraw", name="browraw")
            nc.sync.dma_start(out=raw[:], in_=dB[name][:].rearrange("(y x) -> y x", y=1))
            row = consts.tile([1, n], BF16, tag=f"{name}row")
            if scale is None:
                nc.vector.tensor_copy(row[:], raw[:])
            else:
                nc.vector.tensor_scalar_mul(row[:], raw[:], scale)
            return row


        # ---- inputs ------------------------------------------------------
        # Feature-major bf16 copies (textT/imageT) are prepared host-side
        # (pure layout/dtype prep, like the batch sharding itself); the fp32
        # originals are loaded only for the residual blend.
        def load_tT(dramT, seq, dim, tagbase):
            tT_tiles = []
            for j in range(dim // 128):
                t = ttp.tile([128, seq], BF16, tag=f"{tagbase}T{j}", name=f"{tagbase}T{j}")
                nc.sync.dma_start(out=t[:], in_=dramT[j * 128:(j + 1) * 128, :])
                tT_tiles.append(t)
            return tT_tiles

        def load_raw(dram, seq, dim, tagbase, bbc):
            # rawS = (1-f)*x + f*bout_broadcast; the final blend is then a
            # single psum+rawS add with no bias matmuls
            rawS_tiles = []
            for i in range(seq // 128):
                raw = wraw.tile([128, dim], F32, tag="wraw", name="inraw")
                nc.sync.dma_start(out=raw[:], in_=dram[i * 128:(i + 1) * 128, :])
                rawS = rawin.tile([128, dim], F32, tag=f"{tagbase}rawS{i}", name=f"{tagbase}rawS{i}")
                nc.vector.tensor_scalar_mul(rawS[:], raw[:], omf)
                nc.vector.tensor_add(rawS[:], rawS[:], bbc[:])
                rawS_tiles.append(rawS)
            return rawS_tiles

        # staged strictly by need: the attention-A dependency chain first;
        # residual (fp32) inputs and output-projection weights load late.
        textT = load_tT(textT_dram, ST, DT, "tx")
        imageT = load_tT(imageT_dram, SI, DI, "im")
        Wtp = load_w("Wtp", DT, H)
        btp = load_bcol("btp")
        Wip = load_w("Wip", DI, H)
        bip = load_bcol("bip")
        Wq = load_w("Wq", H, H)
        Wk = load_w("Wk", H, H)
        Wv = load_w("Wv", H, H)
        Wo = load_w("Wo", H, H)
        bqh = load_bcol("bq", half=True)   # 0.5*b for the tanh trick
        bkh = load_bcol("bk", half=True)
        bo = load_bcol("bo")
        bvbc = load_bbc("bv", H, 1.0)

        # ---- projections to hidden (feature-major PtT/PiT) ---------------
        def in_proj(tT, W, bias, tagbase, seq):
            out_tiles = [
                ptp.tile([128, seq], BF16, tag=f"{tagbase}{hc}", name=f"{tagbase}{hc}") for hc in range(2)
            ]
            nd = len(W)
            for hc in range(2):
                for qn in range(seq // 512):
                    ps = ps_m.tile([128, 512], F32, tag="m")
                    for dc in range(nd):
                        nc.tensor.matmul(
                            ps[:],
                            W[dc][:, hc * 128:(hc + 1) * 128],
                            tT[dc][:, qn * 512:(qn + 1) * 512],
                            start=(dc == 0),
                            stop=(dc == nd - 1),
                        )
                    nc.vector.tensor_scalar(
                        out_tiles[hc][:, qn * 512:(qn + 1) * 512],
                        ps[:], bias[hc][:], None, ALU.add,
                    )
            return out_tiles

        PtT = in_proj(textT, Wtp, btp, "pt", ST)
        PiT = in_proj(imageT, Wip, bip, "pi", SI)

        # ---- Q/K memberships: sigmoid via 0.5*tanh(x/2)+0.5 --------------
        def qk_proj(srcT, W, bh, tagbase, seq):
            out_tiles = [
                qkp.tile([128, seq], BF16, tag=f"{tagbase}{hc}", name=f"{tagbase}{hc}") for hc in range(2)
            ]
            for hc in range(2):
                for qn in range(seq // 512):
                    ps = ps_m.tile([128, 512], F32, tag="m")
                    for c in range(2):
                        nc.tensor.matmul(
                            ps[:],
                            W[c][:, hc * 128:(hc + 1) * 128],
                            srcT[c][:, qn * 512:(qn + 1) * 512],
                            start=(c == 0),
                            stop=(c == 1),
                        )
                    th = sigp.tile([128, 512], F32, tag="sig")
                    nc.scalar.activation(th[:], ps[:], AF.Tanh, bias=bh[hc][:], scale=0.5)
                    nc.vector.tensor_scalar(
                        out_tiles[hc][:, qn * 512:(qn + 1) * 512],
                        th[:], 0.5, 0.5, ALU.mult, ALU.add,
                    )
            return out_tiles

        QtT = qk_proj(PtT, Wq, bqh, "qt", ST)
        KiT = qk_proj(PiT, Wk, bkh, "ki", SI)

        # ---- V natural layout, ones-augmented per head --------------------
        def v_proj(srcT, tagbase, seq):
            tiles = []
            for kc in range(seq // 128):
                va = vaugp.tile([128, NH * VSTR], BF16, tag=f"{tagbase}{kc}")
                # ones column per head (read as the 65th weight column)
                nc.vector.memset(
                    va[:].rearrange("p (h c) -> p h c", c=VSTR)[:, :, DH:DH + 1], 1.0
                )
                ps = ps_m.tile([128, H], F32, tag="m")
                for c in range(2):
                    nc.tensor.matmul(
                        ps[:],
                        srcT[c][:, kc * 128:(kc + 1) * 128],
                        Wv[c][:],
                        start=(c == 0),
                        stop=(c == 1),
                    )
                nc.vector.tensor_add(
                    va[:].rearrange("p (h c) -> p h c", c=VSTR)[:, :, 0:DH],
                    ps[:].rearrange("p (h d) -> p h d", d=DH),
                    bvbc[:].rearrange("p (h d) -> p h d", d=DH),
                )
                tiles.append(va)
            return tiles

        Vi = v_proj(PiT, "vi", SI)

        # ---- one cross attention ------------------------------------------
        def attn(QT, KT, Vaug, sq, sk, tagbase, finalize=None):
            """QT: [2][128, sq] query memberships (feature-major)
            KT: [2][128, sk]; Vaug: [sk/128][128, NH*VSTR].
            Returns enhT [2][128, sq] bf16 = (softmax(QK^T/8) Vaug) @ Wo + bo."""
            nkc = sk // 128
            nqc = sq // 128
            O = {}
            for qc in range(nqc):
                O[qc] = opool.tile([128, H], BF16, tag="o", name="o")
            for h in range(NH):
                hc, ro = h // 2, (h % 2) * DH
                Es = []
                for kc in range(nkc):
                    ps = ps_s.tile([128, 1024], F32, tag="s")
                    for qn in range(sq // 512):
                        nc.tensor.matmul(
                            ps[:, qn * 512:(qn + 1) * 512],
                            KT[hc][ro:ro + DH, kc * 128:(kc + 1) * 128],
                            QT[hc][ro:ro + DH, qn * 512:(qn + 1) * 512],
                            start=True, stop=True,
                        )
                    e = epool.tile([128, 1024], BF16, tag="e")
                    nc.scalar.activation(e[:], ps[:], AF.Exp, scale=0.125)
                    Es.append(e)
                for qc in range(nqc):
                    pv = ps_m.tile([128, DH + 1], F32, tag="m", name="pv")
                    for kc in range(nkc):
                        nc.tensor.matmul(
                            pv[:],
                            Es[kc][:, qc * 128:(qc + 1) * 128],
                            Vaug[kc][:, h * VSTR:h * VSTR + DH + 1],
                            start=(kc == 0),
                            stop=(kc == nkc - 1),
                        )
                    rec = smallp.tile([128, 1], F32, tag="rec")
                    nc.vector.reciprocal(rec[:], pv[:, DH:DH + 1])
                    nc.vector.tensor_scalar(
                        O[qc][:, h * DH:(h + 1) * DH],
                        pv[:, 0:DH], rec[:], None, ALU.mult,
                    )
            # transpose O -> OT (feature-major) for the Wo projection
            OT = [otp.tile([128, sq], BF16, tag="ot", name="ot") for _ in range(2)]
            for qc in range(nqc):
                for j in range(2):
                    pst = ps_m.tile([128, 128], BF16, tag="m", name="pst")
                    nc.tensor.transpose(
                        pst[:], O[qc][:, j * 128:(j + 1) * 128], identity[:]
                    )
                    nc.vector.tensor_copy(OT[j][:, qc * 128:(qc + 1) * 128], pst[:])
            # enhT = Wo^T @ OT + bo (qn-outer so a finalize callback can
            # consume each half as soon as it exists)
            enhT = [
                enhp.tile([128, sq], BF16, tag=f"{tagbase}{hc}", name=f"{tagbase}{hc}") for hc in range(2)
            ]
            for qn in range(sq // 512):
                for hc in range(2):
                    ps = ps_m.tile([128, 512], F32, tag="m")
                    for c in range(2):
                        nc.tensor.matmul(
                            ps[:],
                            Wo[c][:, hc * 128:(hc + 1) * 128],
                            OT[c][:, qn * 512:(qn + 1) * 512],
                            start=(c == 0),
                            stop=(c == 1),
                        )
                    nc.vector.tensor_scalar(
                        enhT[hc][:, qn * 512:(qn + 1) * 512],
                        ps[:], bo[hc][:], None, ALU.add,
                    )
                if finalize is not None:
                    finalize(enhT, qn)
            return enhT

        enhA = attn(QtT, KiT, Vi, ST, SI, "ea")   # text enhanced

        # B-side projections + late loads trace after attn A so the scheduler
        # fills attn-A's engine gaps with them
        KtT = qk_proj(PtT, Wk, bkh, "kt", ST)
        QiT = qk_proj(PiT, Wq, bqh, "qi", SI)
        Vt = v_proj(PtT, "vt", ST)
        btobc = load_bbc("bto", DT, f)
        text_raw = load_raw(text, ST, DT, "tx", btobc)
        Wto = load_w("Wto", H, DT, scale=f)
        biobc = load_bbc("bio", DI, f)
        image_raw = load_raw(image, SI, DI, "im", biobc)
        Wio = load_w("Wio", H, DI, scale=f)

        # ---- final: out = (1-f)*raw + f*(enh @ Wout + bout) ---------------
        # Wto/Wio and bto/bio were pre-scaled by r=f/(1-f), so
        # out = (psum + raw) * (1-f).
        def final_out(enhT, Wout, raw_tiles, dram_out, dim, qcs):
            nnt = (dim + 511) // 512
            for qc in qcs:
                ps = ps_s.tile([128, dim], F32, tag="s")
                for nt in range(nnt):
                    lo = nt * 512
                    hi = min(dim, lo + 512)
                    for c in range(2):
                        nc.tensor.matmul(
                            ps[:, lo:hi],
                            enhT[c][:, qc * 128:(qc + 1) * 128],
                            Wout[c][:, lo:hi],
                            start=(c == 0),
                            stop=(c == 1),
                        )
                ob = outp.tile([128, dim], F32, tag="ob")
                nc.vector.tensor_add(ob[:], ps[:], raw_tiles[qc][:])
                nc.sync.dma_start(
                    out=dram_out[qc * 128:(qc + 1) * 128, :], in_=ob[:]
                )

        final_out(enhA, Wto, text_raw, text_out, DT, range(ST // 128))
        enhB = attn(
            QiT, KtT, Vt, SI, ST, "eb",
            finalize=lambda eT, qn: final_out(
                eT, Wio, image_raw, image_out, DI, range(qn * 4, qn * 4 + 4)
            ),
        )   # image enhanced




_CACHE = {}


def _get_nc(f: float) -> bass.Bass:
    key = round(float(f), 9)
    if key not in _CACHE:
        _CACHE[key] = _build(float(f))
    return _CACHE[key]


def make_in_maps(inputs) -> list:
    import ml_dtypes
    text = np.ascontiguousarray(np.asarray(inputs["text_features"], np.float32))
    image = np.ascontiguousarray(np.asarray(inputs["image_features"], np.float32))
    B = text.shape[0]
    shared = {}
    for name in ["Wtp", "Wip", "Wq", "Wk", "Wv", "Wo", "Wto", "Wio",
                 "btp", "bip", "bq", "bk", "bv", "bo", "bto", "bio"]:
        shared[name] = np.ascontiguousarray(np.asarray(inputs[name], np.float32))
    shared["ident128"] = np.eye(128, dtype=np.float32).astype(ml_dtypes.bfloat16)
    in_maps = []
    for b in range(B):
        m = dict(shared)
        m["text"] = text[b]
        m["image"] = image[b]
        m["textT"] = np.ascontiguousarray(text[b].T).astype(ml_dtypes.bfloat16)
        m["imageT"] = np.ascontiguousarray(image[b].T).astype(ml_dtypes.bfloat16)
        in_maps.append(m)
    return in_maps


def kernel(**inputs) -> tuple:
    B = np.asarray(inputs["text_features"]).shape[0]
    fw = float(np.asarray(inputs["fusion_weight"], np.float32))
    f = float(1.0 / (1.0 + np.exp(-fw)))

    nc = _get_nc(f)
    in_maps = make_in_maps(inputs)

    res = run_bass_kernel_spmd(nc, in_maps, core_ids=list(range(B)))
    text_final = np.stack([res.results[b]["text_final"] for b in range(B)])
    image_final = np.stack([res.results[b]["image_final"] for b in range(B)])
    return text_final, image_final
